# revision 31
# baseline (speedup 1.0000x reference)
"""CrossAttention2D Trainium2 kernel (bf16 compute).

Sharding: data-parallel over batch. B=8 -> one batch element per NeuronCore,
no collectives. Weights replicated; host pre-transposes and casts to bf16.

Per-core math (C=512, Ccross=768, N=1024, 8 heads x 64):
  Q = Wq @ x_b          [C, N]   (lhsT = WqT tiles, bf16)
  K = Wk @ y_b          [C, N]
  VT = (Wv @ y_b).T     [N, C]   (lhsT = y tiles, rhs = WvT; bias via K=1 matmul)
  per head pair ph (heads on PE rows 0-63 / 64-127, row-tiled concurrent MMs):
    ST = K_h^T . Q_h              [k, q] tiles (K=64 matmuls)
    ET = exp(0.125 * ST)          (ScalarE, free=1024 per instr, PSUM -> SBUF bf16)
    OT_aug = [VT_h | 1].T @ ET    [65, q] in [65,512] accumulation groups
    PE-transpose -> [q, 65]; DVE: O = OT[:, :64] * (1/OT[:, 64])
  quirk: out_flat[h*64+r, j*64+d] = O_h[16r+j, d]  (SBUF->SBUF DMAs)
  out = Wo @ quirk + bo  [C, N] fp32

Schedule: the scores+exp loop is ScalarE-bound; AV bursts + transposes +
tail of the previous pair and projections of the next pair are interleaved
into the PE stream to keep the HAM clock-gate warm. A dummy-matmul warmup
runs during the input-DMA head. PSUM: psS 2x[128,1024] (4 banks) +
psX 4x[128,512]-class shared by AV/transpose/proj (4 banks) = 8.
"""

import numpy as np

import concourse.bass as bass
import concourse.mybir as mybir
import concourse.tile as tile
from concourse import bacc
from concourse.bass_utils import run_bass_kernel_spmd
from concourse.masks import make_identity

P = 128
C = 512          # d_embed
CC = 768         # d_cross
N = 1024         # H*W = 32*32
NH = 8
DH = 64
CT = C // P      # 4
CCT = CC // P    # 6
QT = N // P      # 8
HW = 32
B = 8
F32 = mybir.dt.float32
BF16 = mybir.dt.bfloat16

_CACHE = {}


def _build_nc():
    nc = bacc.Bacc("TRN2", target_bir_lowering=False, debug=False, num_devices=B)

    x = nc.dram_tensor("x", [C, N], BF16, kind="ExternalInput")
    y = nc.dram_tensor("y", [CC, N], BF16, kind="ExternalInput")
    wqT = nc.dram_tensor("wqT", [C, C], BF16, kind="ExternalInput")
    wkT = nc.dram_tensor("wkT", [CC, C], BF16, kind="ExternalInput")
    wvT = nc.dram_tensor("wvT", [CC, C], BF16, kind="ExternalInput")
    woT = nc.dram_tensor("woT", [C, C], BF16, kind="ExternalInput")
    bq = nc.dram_tensor("bq", [C], F32, kind="ExternalInput")
    bk = nc.dram_tensor("bk", [C], F32, kind="ExternalInput")
    bv = nc.dram_tensor("bv", [C], BF16, kind="ExternalInput")
    bo = nc.dram_tensor("bo", [C], F32, kind="ExternalInput")
    out = nc.dram_tensor("out", [C, N], F32, kind="ExternalOutput")

    EXP = mybir.ActivationFunctionType.Exp

    with tile.TileContext(nc) as tc:
        with (
            tc.tile_pool(name="const", bufs=1) as constp,
            tc.tile_pool(name="big", bufs=1) as bigp,
            tc.tile_pool(name="et", bufs=1) as etp,
            tc.tile_pool(name="ot", bufs=4) as otp,
            tc.tile_pool(name="rcp", bufs=4) as rcpp,
            tc.tile_pool(name="ev", bufs=4) as evp,
            tc.tile_pool(name="psS", bufs=2, space="PSUM") as psS,
            tc.tile_pool(name="psP", bufs=2, space="PSUM") as psP,
            tc.tile_pool(name="psX", bufs=2, space="PSUM") as psX,
        ):
            # ---- constants ----
            junk_bf = constp.tile([P, P], BF16, name="junk", tag="junk")
            nc.gpsimd.memset(junk_bf[:], 0.125)
            ident = constp.tile([P, P], F32, name="ident", tag="ident")
            make_identity(nc, ident)
            ones_bf = constp.tile([1, P], BF16, name="ones_bf", tag="ones_bf")
            nc.vector.tensor_scalar(
                ones_bf[:], ident[0:1, :], 0.0, 1.0,
                mybir.AluOpType.mult, mybir.AluOpType.add,
            )
            bq_sb = constp.tile([P, CT], F32, name="bq", tag="bq")
            bk_sb = constp.tile([P, CT], F32, name="bk", tag="bk")
            bo_sb = constp.tile([P, CT], F32, name="bo", tag="bo")
            bv_sb = constp.tile([1, C], BF16, name="bv", tag="bv")

            # ---- PE + ACT warmup (runs while input DMAs land) ----
            # ~36 dummy matmuls keep the PE busy >3.4us so the HAM clock
            # gate reaches 8/8 before the first real projection; one junk
            # exp pulls the ACT table load off the critical path.
            psw = psP.tile([P, P], F32, name="psw", tag="psp")
            for _ in range(40):
                nc.tensor.matmul(psw[:], junk_bf[:], junk_bf[:],
                                 start=True, stop=True)

            def pad(n):
                # dependency-free LDWEIGHTS keep the PE HAM activity monitor
                # busy through ScalarE-bound stretches (junk loads are safe:
                # every real matmul self-loads its own weights)
                for _ in range(n):
                    nc.tensor.ldweights(junk_bf[:])

            # ---- weight / activation loads (per-ktile DMAs for queue spread) ----
            x3 = x.rearrange("(t p) n -> p t n", p=P)
            y3 = y.rearrange("(t p) n -> p t n", p=P)
            wq3 = wqT.rearrange("(t p) m -> p t m", p=P)
            wk3 = wkT.rearrange("(t p) m -> p t m", p=P)
            wv3 = wvT.rearrange("(t p) m -> p t m", p=P)
            wo3 = woT.rearrange("(t p) m -> p t m", p=P)

            x_sb = [bigp.tile([P, N], BF16, name=f"x{t}", tag=f"x{t}") for t in range(CT)]
            y_sb = [bigp.tile([P, N], BF16, name=f"y{t}", tag=f"y{t}") for t in range(CCT)]
            wq_sb = [bigp.tile([P, C], BF16, name=f"wq{t}", tag=f"wq{t}") for t in range(CT)]
            wk_sb = [bigp.tile([P, C], BF16, name=f"wk{t}", tag=f"wk{t}") for t in range(CCT)]
            wv_sb = [bigp.tile([P, C], BF16, name=f"wv{t}", tag=f"wv{t}") for t in range(CCT)]
            wo_sb = [bigp.tile([P, C], BF16, name=f"wo{t}", tag=f"wo{t}") for t in range(CT)]
            # head loads: x/wq on sync (shallow per-kt gating for Q-proj);
            # y/wk striped across both queues. Everything needed after the
            # first exp stays OFF the ACT queue (its instruction stream must
            # be free for the exp chain).
            nc.sync.dma_start(bq_sb[:], bq.rearrange("(o p) -> p o", p=P))
            nc.sync.dma_start(bk_sb[:], bk.rearrange("(o p) -> p o", p=P))
            for t in range(CT):
                nc.sync.dma_start(x_sb[t][:], x3[:, t])
                nc.sync.dma_start(wq_sb[t][:], wq3[:, t])
            qs = [nc.scalar, nc.scalar, nc.scalar, nc.sync, nc.sync, nc.sync]
            for t in range(CCT):
                qs[t].dma_start(y_sb[t][:], y3[:, t])
                qs[t].dma_start(wk_sb[t][:], wk3[:, t])
            # junk exp: ACT table load lands right after y/wk, before the
            # first real exp needs it
            junk_et = constp.tile([P, DH], BF16, name="junk_et", tag="junk_et")
            nc.scalar.activation(junk_et[:], junk_bf[:, 0:DH], EXP, scale=0.125)
            for t in range(CCT):
                nc.sync.dma_start(wv_sb[t][:], wv3[:, t])
            nc.sync.dma_start(bv_sb[:], bv[None, :])
            for t in range(CT):
                nc.sync.dma_start(wo_sb[t][:], wo3[:, t])
            nc.sync.dma_start(bo_sb[:], bo.rearrange("(o p) -> p o", p=P))

            q_sb = [bigp.tile([P, N], BF16, name=f"q{t}", tag=f"q{t}") for t in range(CT)]
            k_sb = [bigp.tile([P, N], BF16, name=f"k{t}", tag=f"k{t}") for t in range(CT)]
            # VT buffer: per n-tile, cols laid out [h][65] with col h*65+64 == 1.0
            vt_sb = [bigp.tile([P, NH * (DH + 1)], BF16, name=f"vt{t}", tag=f"vt{t}")
                     for t in range(QT)]
            for t in range(QT):
                nc.gpsimd.memset(vt_sb[t][:], 1.0)

            qk_sb = [bigp.tile([P, N], BF16, name=f"qk{t}", tag=f"qk{t}")
                     for t in range(CT)]
            # ET tiles: double-buffered across pair parity
            et_t = [[[etp.tile([P, N], BF16, name=f"et{par}_{hh}_{kt}",
                               tag=f"et{par}_{hh}_{kt}")
                      for kt in range(QT)] for hh in range(2)] for par in range(2)]
            oa_t = [bigp.tile([P, QT, 2 * DH], BF16, name=f"oa{par}", tag=f"oa{par}")
                    for par in range(2)]

            # ---- projection helpers ----
            def qk_proj_half(ct, dst, w_tiles, src_tiles, nkt, bias_sb, half):
                ps = psP.tile([P, 512], F32, name="ps", tag="psp")
                for kt in range(nkt):
                    nc.tensor.matmul(
                        ps[:],
                        w_tiles[kt][:, ct * P:(ct + 1) * P],
                        src_tiles[kt][:, half * 512:(half + 1) * 512],
                        start=(kt == 0),
                        stop=(kt == nkt - 1),
                    )
                nc.vector.tensor_scalar_add(
                    dst[:, half * 512:(half + 1) * 512], ps[:], bias_sb[:, ct:ct + 1]
                )

            def qk_proj(ct, dst, w_tiles, src_tiles, nkt, bias_sb):
                for half in range(2):
                    qk_proj_half(ct, dst, w_tiles, src_tiles, nkt, bias_sb, half)

            def vt_proj(nt):
                ps = psP.tile([P, 512], F32, name="ps", tag="psp")
                for kt in range(CCT):
                    nc.tensor.matmul(
                        ps[:],
                        y_sb[kt][:, nt * P:(nt + 1) * P],
                        wv_sb[kt][:],
                        start=(kt == 0),
                        stop=False,
                    )
                nc.tensor.matmul(ps[:], ones_bf[:], bv_sb[:], start=False, stop=True)
                # scatter into [h][0:64] slots (col h*65+64 stays 1.0)
                nc.vector.tensor_copy(
                    out=vt_sb[nt].rearrange("p (h e) -> p h e", e=DH + 1)[:, :, 0:DH],
                    in_=ps.rearrange("p (h d) -> p h d", d=DH),
                )

            # ---- deferred-work chunks for a completed pair ----
            # AV accumulation bursts, OT transposes + normalize + quirk DMA.
            COPY = mybir.ActivationFunctionType.Copy

            def av_half(ph, hh, qh, st, ot_dst, kts, use_act=False):
                """Half of an AV accumulation group (4 of 8 k-tiles); the
                PSUM tile is allocated in the first half and carried in st."""
                par = ph % 2
                g = (2 * ph + hh) * (DH + 1)
                if kts[0] == 0:
                    st["otps"] = psX.tile([DH + 1, 512], F32, name="otp", tag="otp")
                otps = st["otps"]
                for kt in kts:
                    nc.tensor.matmul(
                        otps[:],
                        vt_sb[kt][:, g:g + DH + 1],
                        et_t[par][hh][kt][:, qh * 512:(qh + 1) * 512],
                        start=(kt == 0),
                        stop=(kt == QT - 1),
                    )
                if kts[-1] == QT - 1:
                    if use_act and (hh + qh) % 2:  # last retire: split DVE/ACT
                        nc.scalar.activation(ot_dst[:], otps[:], COPY)
                    else:
                        nc.vector.tensor_copy(out=ot_dst[:], in_=otps[:])

            def tail_sub(ph, hh, qh, ot_src, qqs, use_act=False):
                par = ph % 2
                oa = oa_t[par]
                for qq in qqs:
                    qt = qh * 4 + qq
                    tps = psP.tile([P, DH + 1], F32, name="tps", tag="psp")
                    nc.tensor.transpose(
                        tps[:],
                        ot_src[:, qq * P:(qq + 1) * P],
                        ident[0:DH + 1, 0:DH + 1],
                    )
                    rcp = rcpp.tile([P, 1], F32, name="rcp", tag="rcp")
                    nc.vector.reciprocal(rcp[:], tps[:, DH:DH + 1])
                    if use_act and qq % 2:  # last retire: split normalize DVE/ACT
                        nc.scalar.activation(
                            oa[:, qt, hh * DH:(hh + 1) * DH], tps[:, 0:DH],
                            COPY, scale=rcp[:],
                        )
                    else:
                        nc.vector.tensor_scalar_mul(
                            oa[:, qt, hh * DH:(hh + 1) * DH], tps[:, 0:DH], rcp[:]
                        )
                    # quirk shuffle: qk[ph*128 + hh*64 + 8*qt + rr, j*64+d]
                    #   = O[128*qt + 16*rr + j, (2ph+hh)*64 + d]
                    nc.sync.dma_start(
                        qk_sb[ph][64 * hh + 8 * qt: 64 * hh + 8 * qt + 8, :],
                        oa[:, qt, hh * DH:(hh + 1) * DH],
                    )

            def retire_chunks(ph, hh, use_act=False):
                """Fine-grained PE-filler chunks retiring head hh of pair ph:
                2 AV half-bursts + 2 transpose/normalize subs per q-half,
                staggered so nothing waits on its own copy."""
                chunks = []
                for qh in range(2):
                    ot_sb = otp.tile([DH + 1, 512], F32, name="otsb", tag="otsb")
                    st = {}
                    chunks.append((lambda ph=ph, hh=hh, qh=qh, st=st, o=ot_sb:
                                   av_half(ph, hh, qh, st, o, [0, 1, 2, 3], use_act)))
                    chunks.append((lambda ph=ph, hh=hh, qh=qh, st=st, o=ot_sb:
                                   av_half(ph, hh, qh, st, o, [4, 5, 6, 7], use_act)))
                    chunks.append((lambda ph=ph, hh=hh, qh=qh, o=ot_sb:
                                   tail_sub(ph, hh, qh, o, [0, 1], use_act)))
                    chunks.append((lambda ph=ph, hh=hh, qh=qh, o=ot_sb:
                                   tail_sub(ph, hh, qh, o, [2, 3], use_act)))
                # stagger: avq0a, avq0b, avq1a, t0a, avq1b, t0b, t1a, t1b
                return [chunks[0], chunks[1], chunks[4], chunks[2], chunks[5],
                        chunks[3], chunks[6], chunks[7]]

            # ---- attention phase1 (scores+exp) over 16 (head, kt) slots ----
            # h0's 8 slots run first, so h0 of THIS pair can be retired as
            # filler during the h1 half — only h1 of the last pair is left
            # after the loop. fill_a feeds slots 0-7, fill_b slots 8-15.
            def phase1(ph, fill_a, fill_b, pad_per_slot=0):
                par = ph % 2
                ia = ib = 0
                for slot in range(16):
                    hh, kt = slot // QT, slot % QT
                    if pad_per_slot:
                        pad(pad_per_slot)
                    bp = hh * DH
                    sps = psS.tile([P, N], F32, name="sps", tag="sps")
                    for half in range(2):
                        nc.tensor.matmul(
                            sps[:, half * 512:(half + 1) * 512],
                            k_sb[ph][bp:bp + DH, kt * P:(kt + 1) * P],
                            q_sb[ph][bp:bp + DH, half * 512:(half + 1) * 512],
                            start=True,
                            stop=True,
                        )
                    nc.scalar.activation(
                        et_t[par][hh][kt][:], sps[:], EXP, scale=0.125,
                    )
                    if slot < 8:
                        want = (slot + 1) * len(fill_a) // QT
                        while ia < want:
                            fill_a[ia]()
                            ia += 1
                    else:
                        while ia < len(fill_a):
                            fill_a[ia]()
                            ia += 1
                        want = (slot - 7) * len(fill_b) // QT
                        while ib < want:
                            fill_b[ib]()
                            ib += 1
                while ib < len(fill_b):
                    fill_b[ib]()
                    ib += 1

            # ---- main schedule ----
            qk_proj(0, q_sb[0], wq_sb, x_sb, CT, bq_sb)
            qk_proj(0, k_sb[0], wk_sb, y_sb, CCT, bk_sb)

            def pq_half(p, half):
                return lambda: qk_proj_half(p, q_sb[p], wq_sb, x_sb, CT, bq_sb, half)

            def pk_half(p, half):
                return lambda: qk_proj_half(p, k_sb[p], wk_sb, y_sb, CCT, bk_sb, half)

            for ph in range(NH // 2):
                if ph == 0:
                    fill_a = [lambda nt=nt: vt_proj(nt) for nt in range(4)]
                    fill_a += [pq_half(1, 0), pq_half(1, 1)]
                    fill_b = [lambda nt=nt: vt_proj(nt) for nt in range(4, 8)]
                    fill_b += retire_chunks(0, 0)
                    fill_b += [pk_half(1, 0), pk_half(1, 1)]
                else:
                    fill_a = retire_chunks(ph - 1, 1)
                    fill_b = retire_chunks(ph, 0)
                    if ph + 1 < NH // 2:
                        fill_a = fill_a[:4] + [pq_half(ph + 1, 0)] + fill_a[4:] \
                            + [pq_half(ph + 1, 1)]
                        fill_b = fill_b[:4] + [pk_half(ph + 1, 0)] + fill_b[4:] \
                            + [pk_half(ph + 1, 1)]
                phase1(ph, fill_a, fill_b,
                       pad_per_slot=(1 if ph < 2 else (2 if ph == 2 else 5)))

            # retire h1 of the last pair on the now-idle ScalarE (padded to
            # keep the HAM gate warm)
            for ch in retire_chunks(NH // 2 - 1, 1, use_act=True):
                pad(6)
                ch()

            # ---- output projection ----
            # (kt ascending: only the kt=3 matmul waits on pair-3's quirk
            # DMAs, so kt 0-2 of the first groups run during the tail)
            out3 = out.rearrange("(t p) n -> p t n", p=P)
            for ct in range(CT):
                for half in range(2):
                    ps = psP.tile([P, 512], F32, name="ps", tag="psp")
                    for kt in range(CT):
                        nc.tensor.matmul(
                            ps[:],
                            wo_sb[kt][:, ct * P:(ct + 1) * P],
                            qk_sb[kt][:, half * 512:(half + 1) * 512],
                            start=(kt == 0),
                            stop=(kt == CT - 1),
                        )
                    ev = evp.tile([P, 512], F32, name="ev", tag="ev")
                    nc.vector.tensor_scalar_add(ev[:], ps[:], bo_sb[:, ct:ct + 1])
                    eng = nc.sync if (ct + half) % 2 == 0 else nc.scalar
                    eng.dma_start(out3[:, ct, half * 512:(half + 1) * 512], ev[:])
                    pad(4)

    nc.compile()
    return nc


def kernel(**inputs) -> np.ndarray:
    import ml_dtypes
    bf = ml_dtypes.bfloat16

    x = np.ascontiguousarray(np.asarray(inputs["x"], dtype=np.float32).astype(bf))
    y = np.ascontiguousarray(np.asarray(inputs["y"], dtype=np.float32).astype(bf))
    wqT = np.ascontiguousarray(np.asarray(inputs["w_q"], dtype=np.float32).T.astype(bf))
    wkT = np.ascontiguousarray(np.asarray(inputs["w_k"], dtype=np.float32).T.astype(bf))
    wvT = np.ascontiguousarray(np.asarray(inputs["w_v"], dtype=np.float32).T.astype(bf))
    woT = np.ascontiguousarray(np.asarray(inputs["w_o"], dtype=np.float32).T.astype(bf))
    bq = np.ascontiguousarray(np.asarray(inputs["b_q"], dtype=np.float32))
    bk = np.ascontiguousarray(np.asarray(inputs["b_k"], dtype=np.float32))
    bv = np.ascontiguousarray(np.asarray(inputs["b_v"], dtype=np.float32).astype(bf))
    bo = np.ascontiguousarray(np.asarray(inputs["b_o"], dtype=np.float32))

    if "nc" not in _CACHE:
        _CACHE["nc"] = _build_nc()
    nc = _CACHE["nc"]

    in_maps = []
    for b in range(B):
        in_maps.append({
            "x": np.ascontiguousarray(x[b].reshape(C, N)),
            "y": np.ascontiguousarray(y[b].reshape(CC, N)),
            "wqT": wqT, "wkT": wkT, "wvT": wvT, "woT": woT,
            "bq": bq, "bk": bk, "bv": bv, "bo": bo,
        })
    res = run_bass_kernel_spmd(nc, in_maps, core_ids=list(range(B)))
    return np.stack([res.results[b]["out"].reshape(C, HW, HW) for b in range(B)])


# revision 33
# speedup vs baseline: 1.2149x; 1.2149x over previous
"""CrossAttention2D Trainium2 kernel (bf16 compute).

Sharding: data-parallel over batch. B=8 -> one batch element per NeuronCore,
no collectives. Weights replicated; host pre-transposes and casts to bf16.

Per-core math (C=512, Ccross=768, N=1024, 8 heads x 64):
  Q = Wq @ x_b          [C, N]   (lhsT = WqT tiles, bf16)
  K = Wk @ y_b          [C, N]
  VT = (Wv @ y_b).T     [N, C]   (lhsT = y tiles, rhs = WvT; bias via K=1 matmul)
  per head pair ph (heads on PE rows 0-63 / 64-127, row-tiled concurrent MMs):
    ST = K_h^T . Q_h              [k, q] tiles (K=64 matmuls)
    ET = exp(0.125 * ST)          (ScalarE, free=1024 per instr, PSUM -> SBUF bf16)
    OT_aug = [VT_h | 1].T @ ET    [65, q] in [65,512] accumulation groups
    PE-transpose -> [q, 65]; DVE: O = OT[:, :64] * (1/OT[:, 64])
  quirk: out_flat[h*64+r, j*64+d] = O_h[16r+j, d]  (SBUF->SBUF DMAs)
  out = Wo @ quirk + bo  [C, N] fp32

Schedule: the scores+exp loop is ScalarE-bound; AV bursts + transposes +
tail of the previous pair and projections of the next pair are interleaved
into the PE stream to keep the HAM clock-gate warm. A dummy-matmul warmup
runs during the input-DMA head. PSUM: psS 2x[128,1024] (4 banks) +
psX 4x[128,512]-class shared by AV/transpose/proj (4 banks) = 8.
"""

import numpy as np

import concourse.bass as bass
import concourse.mybir as mybir
import concourse.tile as tile
from concourse import bacc
from concourse.bass_utils import run_bass_kernel_spmd
from concourse.masks import make_identity

P = 128
C = 512          # d_embed
CC = 768         # d_cross
N = 1024         # H*W = 32*32
NH = 8
DH = 64
CT = C // P      # 4
CCT = CC // P    # 6
QT = N // P      # 8
HW = 32
B = 8
F32 = mybir.dt.float32
BF16 = mybir.dt.bfloat16

_CACHE = {}


def _build_nc():
    nc = bacc.Bacc("TRN2", target_bir_lowering=False, debug=False, num_devices=B)

    x = nc.dram_tensor("x", [C, N], BF16, kind="ExternalInput")
    y = nc.dram_tensor("y", [CC, N], BF16, kind="ExternalInput")
    wqT = nc.dram_tensor("wqT", [C, C], BF16, kind="ExternalInput")
    wkT = nc.dram_tensor("wkT", [CC, C], BF16, kind="ExternalInput")
    wvT = nc.dram_tensor("wvT", [CC, C], BF16, kind="ExternalInput")
    woT = nc.dram_tensor("woT", [C, C], BF16, kind="ExternalInput")
    bq = nc.dram_tensor("bq", [C], F32, kind="ExternalInput")
    bk = nc.dram_tensor("bk", [C], F32, kind="ExternalInput")
    bv = nc.dram_tensor("bv", [C], BF16, kind="ExternalInput")
    bo = nc.dram_tensor("bo", [C], F32, kind="ExternalInput")
    out = nc.dram_tensor("out", [C, N], F32, kind="ExternalOutput")

    EXP = mybir.ActivationFunctionType.Exp

    with tile.TileContext(nc) as tc:
        with (
            tc.tile_pool(name="const", bufs=1) as constp,
            tc.tile_pool(name="big", bufs=1) as bigp,
            tc.tile_pool(name="et", bufs=1) as etp,
            tc.tile_pool(name="ot", bufs=4) as otp,
            tc.tile_pool(name="rcp", bufs=4) as rcpp,
            tc.tile_pool(name="ev", bufs=4) as evp,
            tc.tile_pool(name="psS", bufs=2, space="PSUM") as psS,
            tc.tile_pool(name="psP", bufs=2, space="PSUM") as psP,
            tc.tile_pool(name="psX", bufs=2, space="PSUM") as psX,
        ):
            # ---- constants ----
            junk_bf = constp.tile([P, P], BF16, name="junk", tag="junk")
            nc.gpsimd.memset(junk_bf[:], 0.125)
            ident = constp.tile([P, P], F32, name="ident", tag="ident")
            make_identity(nc, ident)
            ones_bf = constp.tile([1, P], BF16, name="ones_bf", tag="ones_bf")
            nc.vector.tensor_scalar(
                ones_bf[:], ident[0:1, :], 0.0, 1.0,
                mybir.AluOpType.mult, mybir.AluOpType.add,
            )
            bq_sb = constp.tile([P, CT], F32, name="bq", tag="bq")
            bk_sb = constp.tile([P, CT], F32, name="bk", tag="bk")
            bo_sb = constp.tile([P, CT], F32, name="bo", tag="bo")
            bv_sb = constp.tile([1, C], BF16, name="bv", tag="bv")

            # ---- PE + ACT warmup (runs while input DMAs land) ----
            # ~36 dummy matmuls keep the PE busy >3.4us so the HAM clock
            # gate reaches 8/8 before the first real projection; one junk
            # exp pulls the ACT table load off the critical path.
            psw = psP.tile([P, P], F32, name="psw", tag="psp")
            for _ in range(40):
                nc.tensor.matmul(psw[:], junk_bf[:], junk_bf[:],
                                 start=True, stop=True)

            def pad(n):
                # dependency-free LDWEIGHTS keep the PE HAM activity monitor
                # busy through ScalarE-bound stretches (junk loads are safe:
                # every real matmul self-loads its own weights)
                for _ in range(n):
                    nc.tensor.ldweights(junk_bf[:])

            # ---- weight / activation loads (per-ktile DMAs for queue spread) ----
            x3 = x.rearrange("(t p) n -> p t n", p=P)
            y3 = y.rearrange("(t p) n -> p t n", p=P)
            wq3 = wqT.rearrange("(t p) m -> p t m", p=P)
            wk3 = wkT.rearrange("(t p) m -> p t m", p=P)
            wv3 = wvT.rearrange("(t p) m -> p t m", p=P)
            wo3 = woT.rearrange("(t p) m -> p t m", p=P)

            x_sb = [bigp.tile([P, N], BF16, name=f"x{t}", tag=f"x{t}") for t in range(CT)]
            y_sb = [bigp.tile([P, N], BF16, name=f"y{t}", tag=f"y{t}") for t in range(CCT)]
            wq_sb = [bigp.tile([P, C], BF16, name=f"wq{t}", tag=f"wq{t}") for t in range(CT)]
            wk_sb = [bigp.tile([P, C], BF16, name=f"wk{t}", tag=f"wk{t}") for t in range(CCT)]
            wv_sb = [bigp.tile([P, C], BF16, name=f"wv{t}", tag=f"wv{t}") for t in range(CCT)]
            wo_sb = [bigp.tile([P, C], BF16, name=f"wo{t}", tag=f"wo{t}") for t in range(CT)]
            # head loads striped across both HWDGE queues (SP + ACT) in
            # need-order. Everything needed after the first exp stays OFF
            # the ACT queue (its instruction stream must be free for exp).
            nc.sync.dma_start(bq_sb[:], bq.rearrange("(o p) -> p o", p=P))
            nc.sync.dma_start(bk_sb[:], bk.rearrange("(o p) -> p o", p=P))
            qs = [nc.sync, nc.scalar]
            for t in range(CT):
                qs[t % 2].dma_start(x_sb[t][:], x3[:, t])
                qs[t % 2].dma_start(wq_sb[t][:], wq3[:, t])
            for t in range(CCT):
                qs[t % 2].dma_start(y_sb[t][:], y3[:, t])
                qs[t % 2].dma_start(wk_sb[t][:], wk3[:, t])
            # junk exp: ACT table load lands right after y/wk, before the
            # first real exp needs it
            junk_et = constp.tile([P, DH], BF16, name="junk_et", tag="junk_et")
            nc.scalar.activation(junk_et[:], junk_bf[:, 0:DH], EXP, scale=0.125)
            for t in range(CCT):
                nc.sync.dma_start(wv_sb[t][:], wv3[:, t])
            nc.sync.dma_start(bv_sb[:], bv[None, :])
            for t in range(CT):
                nc.sync.dma_start(wo_sb[t][:], wo3[:, t])
            nc.sync.dma_start(bo_sb[:], bo.rearrange("(o p) -> p o", p=P))

            q_sb = [bigp.tile([P, N], BF16, name=f"q{t}", tag=f"q{t}") for t in range(CT)]
            k_sb = [bigp.tile([P, N], BF16, name=f"k{t}", tag=f"k{t}") for t in range(CT)]
            # VT buffer: per n-tile, cols laid out [h][65] with col h*65+64 == 1.0
            vt_sb = [bigp.tile([P, NH * (DH + 1)], BF16, name=f"vt{t}", tag=f"vt{t}")
                     for t in range(QT)]
            for t in range(QT):
                nc.gpsimd.memset(vt_sb[t][:], 1.0)

            qk_sb = [bigp.tile([P, N], BF16, name=f"qk{t}", tag=f"qk{t}")
                     for t in range(CT)]
            # ET tiles: double-buffered across pair parity
            et_t = [[[etp.tile([P, N], BF16, name=f"et{par}_{hh}_{kt}",
                               tag=f"et{par}_{hh}_{kt}")
                      for kt in range(QT)] for hh in range(2)] for par in range(2)]
            oa_t = [bigp.tile([P, QT, 2 * DH], BF16, name=f"oa{par}", tag=f"oa{par}")
                    for par in range(2)]

            # ---- projection helpers ----
            def qk_proj_half(ct, dst, w_tiles, src_tiles, nkt, bias_sb, half):
                ps = psP.tile([P, 512], F32, name="ps", tag="psp")
                for kt in range(nkt):
                    nc.tensor.matmul(
                        ps[:],
                        w_tiles[kt][:, ct * P:(ct + 1) * P],
                        src_tiles[kt][:, half * 512:(half + 1) * 512],
                        start=(kt == 0),
                        stop=(kt == nkt - 1),
                    )
                nc.vector.tensor_scalar_add(
                    dst[:, half * 512:(half + 1) * 512], ps[:], bias_sb[:, ct:ct + 1]
                )

            def qk_proj(ct, dst, w_tiles, src_tiles, nkt, bias_sb):
                for half in range(2):
                    qk_proj_half(ct, dst, w_tiles, src_tiles, nkt, bias_sb, half)

            def vt_proj(nt):
                ps = psP.tile([P, 512], F32, name="ps", tag="psp")
                for kt in range(CCT):
                    nc.tensor.matmul(
                        ps[:],
                        y_sb[kt][:, nt * P:(nt + 1) * P],
                        wv_sb[kt][:],
                        start=(kt == 0),
                        stop=False,
                    )
                nc.tensor.matmul(ps[:], ones_bf[:], bv_sb[:], start=False, stop=True)
                # scatter into [h][0:64] slots (col h*65+64 stays 1.0)
                nc.vector.tensor_copy(
                    out=vt_sb[nt].rearrange("p (h e) -> p h e", e=DH + 1)[:, :, 0:DH],
                    in_=ps.rearrange("p (h d) -> p h d", d=DH),
                )

            # ---- deferred-work chunks for a completed pair ----
            # AV accumulation bursts, OT transposes + normalize + quirk DMA.
            COPY = mybir.ActivationFunctionType.Copy

            def av_half(ph, hh, qh, st, ot_dst, kts, use_act=False):
                """Half of an AV accumulation group (4 of 8 k-tiles); the
                PSUM tile is allocated in the first half and carried in st."""
                par = ph % 2
                g = (2 * ph + hh) * (DH + 1)
                if kts[0] == 0:
                    st["otps"] = psX.tile([DH + 1, 512], F32, name="otp", tag="otp")
                otps = st["otps"]
                for kt in kts:
                    nc.tensor.matmul(
                        otps[:],
                        vt_sb[kt][:, g:g + DH + 1],
                        et_t[par][hh][kt][:, qh * 512:(qh + 1) * 512],
                        start=(kt == 0),
                        stop=(kt == QT - 1),
                    )
                if kts[-1] == QT - 1:
                    if use_act and (hh + qh) % 2:  # last retire: split DVE/ACT
                        nc.scalar.activation(ot_dst[:], otps[:], COPY)
                    else:
                        nc.vector.tensor_copy(out=ot_dst[:], in_=otps[:])

            def tail_sub(ph, hh, qh, ot_src, qqs, use_act=False):
                par = ph % 2
                oa = oa_t[par]
                for qq in qqs:
                    qt = qh * 4 + qq
                    tps = psP.tile([P, DH + 1], F32, name="tps", tag="psp")
                    nc.tensor.transpose(
                        tps[:],
                        ot_src[:, qq * P:(qq + 1) * P],
                        ident[0:DH + 1, 0:DH + 1],
                    )
                    rcp = rcpp.tile([P, 1], F32, name="rcp", tag="rcp")
                    nc.vector.reciprocal(rcp[:], tps[:, DH:DH + 1])
                    if use_act and qq % 2:  # last retire: split normalize DVE/ACT
                        nc.scalar.activation(
                            oa[:, qt, hh * DH:(hh + 1) * DH], tps[:, 0:DH],
                            COPY, scale=rcp[:],
                        )
                    else:
                        nc.vector.tensor_scalar_mul(
                            oa[:, qt, hh * DH:(hh + 1) * DH], tps[:, 0:DH], rcp[:]
                        )
                    # quirk shuffle: qk[ph*128 + hh*64 + 8*qt + rr, j*64+d]
                    #   = O[128*qt + 16*rr + j, (2ph+hh)*64 + d]
                    nc.sync.dma_start(
                        qk_sb[ph][64 * hh + 8 * qt: 64 * hh + 8 * qt + 8, :],
                        oa[:, qt, hh * DH:(hh + 1) * DH],
                    )

            def retire_chunks(ph, hh, use_act=False):
                """Fine-grained PE-filler chunks retiring head hh of pair ph:
                2 AV half-bursts + 2 transpose/normalize subs per q-half,
                staggered so nothing waits on its own copy."""
                chunks = []
                for qh in range(2):
                    ot_sb = otp.tile([DH + 1, 512], F32, name="otsb", tag="otsb")
                    st = {}
                    chunks.append((lambda ph=ph, hh=hh, qh=qh, st=st, o=ot_sb:
                                   av_half(ph, hh, qh, st, o, [0, 1, 2, 3], use_act)))
                    chunks.append((lambda ph=ph, hh=hh, qh=qh, st=st, o=ot_sb:
                                   av_half(ph, hh, qh, st, o, [4, 5, 6, 7], use_act)))
                    chunks.append((lambda ph=ph, hh=hh, qh=qh, o=ot_sb:
                                   tail_sub(ph, hh, qh, o, [0, 1], use_act)))
                    chunks.append((lambda ph=ph, hh=hh, qh=qh, o=ot_sb:
                                   tail_sub(ph, hh, qh, o, [2, 3], use_act)))
                # stagger: avq0a, avq0b, avq1a, t0a, avq1b, t0b, t1a, t1b
                return [chunks[0], chunks[1], chunks[4], chunks[2], chunks[5],
                        chunks[3], chunks[6], chunks[7]]

            # ---- attention phase1 (scores+exp), joint-head kt slots so the
            # h0/h1 score matmuls overlap via PE row-group tiling ----
            def phase1(ph, fillers, pad_per_slot=0):
                par = ph % 2
                fi = 0
                nf = len(fillers)
                for kt in range(QT):
                    if pad_per_slot:
                        pad(pad_per_slot)
                    sps = {hh: psS.tile([P, N], F32, name="sps", tag="sps")
                           for hh in range(2)}
                    for half in range(2):
                        for hh in range(2):  # alternate row groups for concurrency
                            bp = hh * DH
                            nc.tensor.matmul(
                                sps[hh][:, half * 512:(half + 1) * 512],
                                k_sb[ph][bp:bp + DH, kt * P:(kt + 1) * P],
                                q_sb[ph][bp:bp + DH, half * 512:(half + 1) * 512],
                                start=True,
                                stop=True,
                            )
                    for hh in range(2):
                        nc.scalar.activation(
                            et_t[par][hh][kt][:], sps[hh][:], EXP, scale=0.125,
                        )
                    want = (kt + 1) * nf // QT
                    while fi < want:
                        fillers[fi]()
                        fi += 1
                while fi < nf:
                    fillers[fi]()
                    fi += 1

            # ---- main schedule ----
            qk_proj(0, q_sb[0], wq_sb, x_sb, CT, bq_sb)
            qk_proj(0, k_sb[0], wk_sb, y_sb, CCT, bk_sb)

            def pq_half(p, half):
                return lambda: qk_proj_half(p, q_sb[p], wq_sb, x_sb, CT, bq_sb, half)

            def pk_half(p, half):
                return lambda: qk_proj_half(p, k_sb[p], wk_sb, y_sb, CCT, bk_sb, half)

            for ph in range(NH // 2):
                if ph == 0:
                    fillers = [lambda nt=nt: vt_proj(nt) for nt in range(QT)]
                    fillers[2:2] = [pq_half(1, 0), pq_half(1, 1)]
                    fillers[6:6] = [pk_half(1, 0), pk_half(1, 1)]
                else:
                    r0 = retire_chunks(ph - 1, 0)
                    r1 = retire_chunks(ph - 1, 1)
                    # interleave both heads' retire chunks + next-pair proj
                    fillers = []
                    for a, b in zip(r0, r1):
                        fillers += [a, b]
                    if ph + 1 < NH // 2:
                        fillers[4:4] = [pq_half(ph + 1, 0)]
                        fillers[9:9] = [pq_half(ph + 1, 1)]
                        fillers[12:12] = [pk_half(ph + 1, 0)]
                        fillers[16:16] = [pk_half(ph + 1, 1)]
                phase1(ph, fillers,
                       pad_per_slot=(2 if ph < 2 else (3 if ph == 2 else 6)))

            # retire the last pair on the now-idle ScalarE (padded to keep
            # the HAM gate warm)
            r0 = retire_chunks(NH // 2 - 1, 0, use_act=True)
            r1 = retire_chunks(NH // 2 - 1, 1, use_act=True)
            for a, b in zip(r0, r1):
                pad(4)
                a()
                pad(4)
                b()

            # ---- output projection ----
            # (kt ascending: only the kt=3 matmul waits on pair-3's quirk
            # DMAs, so kt 0-2 of the first groups run during the tail)
            out3 = out.rearrange("(t p) n -> p t n", p=P)
            for ct in range(CT):
                for half in range(2):
                    ps = psP.tile([P, 512], F32, name="ps", tag="psp")
                    for kt in range(CT):
                        nc.tensor.matmul(
                            ps[:],
                            wo_sb[kt][:, ct * P:(ct + 1) * P],
                            qk_sb[kt][:, half * 512:(half + 1) * 512],
                            start=(kt == 0),
                            stop=(kt == CT - 1),
                        )
                    ev = evp.tile([P, 512], F32, name="ev", tag="ev")
                    nc.vector.tensor_scalar_add(ev[:], ps[:], bo_sb[:, ct:ct + 1])
                    eng = nc.sync if (ct + half) % 2 == 0 else nc.scalar
                    eng.dma_start(out3[:, ct, half * 512:(half + 1) * 512], ev[:])
                    pad(4)

    nc.compile()
    return nc


def kernel(**inputs) -> np.ndarray:
    import ml_dtypes
    bf = ml_dtypes.bfloat16

    x = np.ascontiguousarray(np.asarray(inputs["x"], dtype=np.float32).astype(bf))
    y = np.ascontiguousarray(np.asarray(inputs["y"], dtype=np.float32).astype(bf))
    wqT = np.ascontiguousarray(np.asarray(inputs["w_q"], dtype=np.float32).T.astype(bf))
    wkT = np.ascontiguousarray(np.asarray(inputs["w_k"], dtype=np.float32).T.astype(bf))
    wvT = np.ascontiguousarray(np.asarray(inputs["w_v"], dtype=np.float32).T.astype(bf))
    woT = np.ascontiguousarray(np.asarray(inputs["w_o"], dtype=np.float32).T.astype(bf))
    bq = np.ascontiguousarray(np.asarray(inputs["b_q"], dtype=np.float32))
    bk = np.ascontiguousarray(np.asarray(inputs["b_k"], dtype=np.float32))
    bv = np.ascontiguousarray(np.asarray(inputs["b_v"], dtype=np.float32).astype(bf))
    bo = np.ascontiguousarray(np.asarray(inputs["b_o"], dtype=np.float32))

    if "nc" not in _CACHE:
        _CACHE["nc"] = _build_nc()
    nc = _CACHE["nc"]

    in_maps = []
    for b in range(B):
        in_maps.append({
            "x": np.ascontiguousarray(x[b].reshape(C, N)),
            "y": np.ascontiguousarray(y[b].reshape(CC, N)),
            "wqT": wqT, "wkT": wkT, "wvT": wvT, "woT": woT,
            "bq": bq, "bk": bk, "bv": bv, "bo": bo,
        })
    res = run_bass_kernel_spmd(nc, in_maps, core_ids=list(range(B)))
    return np.stack([res.results[b]["out"].reshape(C, HW, HW) for b in range(B)])
